# revision 18
# baseline (speedup 1.0000x reference)
"""GAT (nn_GAT_76536317214930) on 8 TRN2 NeuronCores.

The reference's attention softmax is dead code; each layer is
    emb = elu(adj @ (x @ Wcat))        with heads concatenated on features,
then out = elu(emb1) @ lin_w + lin_b and log_softmax.

Sharding: rows (destination nodes) of adj split across 8 cores. Each core
holds adjT shard [8192, 1024] (bf16, SBUF-resident, used by both layers),
computes H = x @ Wcat for its own rows, AllGathers H, then accumulates
P^T = H^T-chunks (stationary) @ adjT-chunks (moving) on the PE.

All feature-major [64, n] intermediates; node-major tiles are produced by
matmuls with the feature-major tensor as lhsT (contracting over features),
so no explicit transposes are needed anywhere.
"""
import numpy as np
import ml_dtypes

import jax
from jax.experimental.shard_map import shard_map
from jax.sharding import Mesh, NamedSharding, PartitionSpec

import concourse.bass as bass
import concourse.bacc as bacc
import concourse.mybir as mybir
import concourse.tile as tile
from concourse import bass2jax

NCORES = 8
N = 8192          # nodes
NFEAT = 512       # input features
F = 64            # NHEADS * NHID = 4*16
NCLASS = 40
SH = N // NCORES  # 1024 rows per core
NCH = N // 128    # 64 contraction chunks (global nodes)
SHC = SH // 128   # 8 node tiles per core shard
XCH = NFEAT // 128  # 4 chunks of input features

BF16 = mybir.dt.bfloat16
F32 = mybir.dt.float32
AF = mybir.ActivationFunctionType
ALU = mybir.AluOpType

ADJ_DMA_GROUPS = 16  # adjT loaded in 16 DMAs of 4 chunks (1 MiB each)

_NC_CACHE = {}


def _emit_elu(nc, pool, out_f32, out_alt, src_halves, tag):
    """elu(x) = max(x, exp(min(x, 0)) - 1), per [64, 512] half."""
    for i, (ps, off) in enumerate(src_halves):
        w = ps.shape[-1]
        m_t = pool.tile([F, w], F32, name=f"{tag}_m{i}", tag="elu_m", bufs=2)
        e_t = pool.tile([F, w], F32, name=f"{tag}_e{i}", tag="elu_e", bufs=2)
        nc.vector.tensor_scalar_min(m_t[:], ps, 0.0)
        nc.scalar.activation(e_t[:], m_t[:], AF.Exp)
        nc.vector.tensor_scalar_add(e_t[:], e_t[:], -1.0)
        if out_f32 is not None:
            nc.vector.tensor_tensor(out_f32[:, off:off + w], e_t[:], ps, ALU.max)
        if out_alt is not None:
            nc.vector.tensor_tensor(out_alt[:, off:off + w], e_t[:], ps, ALU.max)


def _emit_body(nc, tc, sb, scratch, pacc, psm, dram, io, rep):
    """One full forward pass. All tiles tagged so reps share SBUF slots."""
    rg = [list(range(NCORES))]
    adjT, xT, w0, w1, lw, emb0T_d, emb1T_d, outp_d, ls_d = io
    r = f"_{rep}"

    def T(pool, shape, dtype, nm, **kw):
        return pool.tile(shape, dtype, name=nm + r, tag=nm, **kw)

    # ---- persistent SBUF tensors ----
    adjT_sb = T(sb, [128, NCH, SH], BF16, "adjT_sb")
    xT_sb = T(sb, [128, XCH, SH], BF16, "xT_sb")
    w0_sb = T(sb, [128, XCH, F], BF16, "w0_sb")
    w1_sb = T(sb, [F, F], BF16, "w1_sb")
    lw_sb = T(sb, [F + 1, NCLASS], BF16, "lw_sb")
    h0_sb = T(sb, [128, NCH * F], BF16, "h0_sb")
    h1_sb = T(sb, [128, NCH * F], BF16, "h1_sb")
    emb0T = T(sb, [F, SH], F32, "emb0T")
    emb0Tb = T(sb, [F, SH], BF16, "emb0Tb")
    emb1T = T(sb, [F, SH], F32, "emb1T")
    e2T = T(sb, [F + 1, SH], BF16, "e2T")
    h0loc = T(sb, [128, SHC, F], BF16, "h0loc")
    h1loc = T(sb, [128, SHC, F], BF16, "h1loc")
    out_sb = T(sb, [128, SHC, NCLASS], F32, "out_sb")
    t_sb = T(sb, [128, SHC, NCLASS], F32, "t_sb")
    ls_sb = T(sb, [128, SHC, NCLASS], F32, "ls_sb")
    mx_sb = T(sb, [128, SHC], F32, "mx_sb")
    s_sb = T(sb, [128, SHC], F32, "s_sb")
    l_sb = T(sb, [128, SHC], F32, "l_sb")

    # ---- input DMAs ----
    adjT_r = adjT.ap().rearrange("(c p) n -> p c n", p=128)
    gsz = NCH // ADJ_DMA_GROUPS
    for g in range(ADJ_DMA_GROUPS):
        nc.sync.dma_start(
            out=adjT_sb[:, g * gsz:(g + 1) * gsz, :],
            in_=adjT_r[:, g * gsz:(g + 1) * gsz, :])
    nc.sync.dma_start(out=xT_sb[:],
                      in_=xT.ap().rearrange("(c p) n -> p c n", p=128))
    nc.sync.dma_start(out=w0_sb[:],
                      in_=w0.ap().rearrange("(c p) f -> p c f", p=128))
    nc.sync.dma_start(out=w1_sb[:], in_=w1[:])
    nc.sync.dma_start(out=lw_sb[:], in_=lw[:])
    nc.vector.memset(e2T[F:F + 1, :], 1.0)

    # ---- H0 = x @ W0cat (node-major tiles), bounce, AllGather ----
    # Bounce/gather keep the raw SBUF layout [p, c_local*F] so every DMA is
    # large-contiguous; global chunk c = rank*SHC + c_local lines up with
    # node order automatically (node = c*128 + p).
    for m in range(SHC):
        ph0 = psm.tile([128, F], F32, name=f"ph0_{m}{r}", tag="psm")
        for kc in range(XCH):
            nc.tensor.matmul(ph0[:], xT_sb[:, kc, m * 128:(m + 1) * 128],
                             w0_sb[:, kc, :],
                             start=(kc == 0), stop=(kc == XCH - 1))
        nc.scalar.activation(h0loc[:, m, :], ph0[:], AF.Copy)
    h0_bounce = T(dram, [128, SHC * F], BF16, "h0_bounce")
    h0_full = T(dram, [NCORES * 128, SHC * F], BF16, "h0_full",
                addr_space="Shared")
    nc.sync.dma_start(out=h0_bounce[:], in_=h0loc[:])
    nc.gpsimd.collective_compute(
        "AllGather", ALU.bypass, replica_groups=rg,
        ins=[h0_bounce[:]], outs=[h0_full[:]])
    nc.sync.dma_start(
        out=h0_sb[:].rearrange("p (g q) -> p g q", g=NCORES),
        in_=h0_full.rearrange("(g p) q -> p g q", p=128))

    # ---- layer 0 big matmul: P0^T[f, n] accumulated over 64 K-chunks ----
    p0a = T(pacc, [F, 512], F32, "acc_a")
    p0b = T(pacc, [F, 512], F32, "acc_b")
    for c in range(NCH):
        st, sp = (c == 0), (c == NCH - 1)
        nc.tensor.matmul(p0a[:], h0_sb[:, c * F:(c + 1) * F],
                         adjT_sb[:, c, 0:512], start=st, stop=sp)
        nc.tensor.matmul(p0b[:], h0_sb[:, c * F:(c + 1) * F],
                         adjT_sb[:, c, 512:SH], start=st, stop=sp)

    # ---- elu -> emb0T (f32 out) + bf16 copy ----
    _emit_elu(nc, scratch, emb0T, None, [(p0a[:], 0), (p0b[:], 512)], "l0" + r)
    nc.sync.dma_start(out=emb0T_d[:], in_=emb0T[:])
    nc.vector.tensor_copy(emb0Tb[:], emb0T[:])

    # ---- H1 tiles = emb0 @ W1cat (node-major), bounce, AllGather ----
    for m in range(SHC):
        ph1 = psm.tile([128, F], F32, name=f"ph1_{m}{r}", tag="psm")
        nc.tensor.matmul(ph1[:], emb0Tb[:, m * 128:(m + 1) * 128],
                         w1_sb[:], start=True, stop=True)
        nc.scalar.activation(h1loc[:, m, :], ph1[:], AF.Copy)
    h1_bounce = T(dram, [128, SHC * F], BF16, "h1_bounce")
    h1_full = T(dram, [NCORES * 128, SHC * F], BF16, "h1_full",
                addr_space="Shared")
    nc.sync.dma_start(out=h1_bounce[:], in_=h1loc[:])
    nc.gpsimd.collective_compute(
        "AllGather", ALU.bypass, replica_groups=rg,
        ins=[h1_bounce[:]], outs=[h1_full[:]])
    nc.sync.dma_start(
        out=h1_sb[:].rearrange("p (g q) -> p g q", g=NCORES),
        in_=h1_full.rearrange("(g p) q -> p g q", p=128))

    # ---- layer 1 big matmul ----
    p1a = pacc.tile([F, 512], F32, name=f"acc_a2{r}", tag="acc_a")
    p1b = pacc.tile([F, 512], F32, name=f"acc_b2{r}", tag="acc_b")
    for c in range(NCH):
        st, sp = (c == 0), (c == NCH - 1)
        nc.tensor.matmul(p1a[:], h1_sb[:, c * F:(c + 1) * F],
                         adjT_sb[:, c, 0:512], start=st, stop=sp)
        nc.tensor.matmul(p1b[:], h1_sb[:, c * F:(c + 1) * F],
                         adjT_sb[:, c, 512:SH], start=st, stop=sp)

    # ---- elu -> emb1T f32; e2 = elu(emb1) bf16 (ones row = bias) ----
    _emit_elu(nc, scratch, emb1T, None, [(p1a[:], 0), (p1b[:], 512)], "l1" + r)
    nc.sync.dma_start(out=emb1T_d[:], in_=emb1T[:])
    _emit_elu(nc, scratch, None, e2T[0:F, :],
              [(emb1T[:, 0:512], 0), (emb1T[:, 512:SH], 512)], "e2" + r)

    # ---- classifier: out = e2 @ [lin_w; lin_b] (node-major) ----
    for m in range(SHC):
        pcls = psm.tile([128, NCLASS], F32, name=f"pcls_{m}{r}", tag="psm")
        nc.tensor.matmul(pcls[:], e2T[:, m * 128:(m + 1) * 128],
                         lw_sb[:], start=True, stop=True)
        nc.scalar.activation(out_sb[:, m, :], pcls[:], AF.Copy)
    nc.sync.dma_start(out=outp_d.ap(), in_=out_sb[:])

    # ---- log_softmax over classes (free axis) ----
    for m in range(SHC):
        nc.vector.tensor_reduce(mx_sb[:, m:m + 1], out_sb[:, m, :],
                                mybir.AxisListType.X, ALU.max)
        nc.vector.tensor_scalar(t_sb[:, m, :], out_sb[:, m, :],
                                mx_sb[:, m:m + 1], None, ALU.subtract)
        e_sm = scratch.tile([128, NCLASS], F32, name=f"e_sm_{m}{r}",
                            tag="e_sm", bufs=2)
        nc.scalar.activation(e_sm[:], t_sb[:, m, :], AF.Exp,
                             accum_out=s_sb[:, m:m + 1])
    nc.scalar.activation(l_sb[:], s_sb[:], AF.Ln)
    for m in range(SHC):
        nc.vector.tensor_scalar(ls_sb[:, m, :], t_sb[:, m, :],
                                l_sb[:, m:m + 1], None, ALU.subtract)
    nc.sync.dma_start(out=ls_d.ap(), in_=ls_sb[:])


def _build(reps=1):
    nc = bacc.Bacc("TRN2", target_bir_lowering=False, debug=False,
                   num_devices=NCORES)

    adjT = nc.dram_tensor("adjT", [N, SH], BF16, kind="ExternalInput")
    xT = nc.dram_tensor("xT", [NFEAT, SH], BF16, kind="ExternalInput")
    w0 = nc.dram_tensor("w0", [NFEAT, F], BF16, kind="ExternalInput")
    w1 = nc.dram_tensor("w1", [F, F], BF16, kind="ExternalInput")
    lw = nc.dram_tensor("lw", [F + 1, NCLASS], BF16, kind="ExternalInput")

    emb0T_d = nc.dram_tensor("emb0T", [F, SH], F32, kind="ExternalOutput")
    emb1T_d = nc.dram_tensor("emb1T", [F, SH], F32, kind="ExternalOutput")
    # raw SBUF layout [p, c_local, class]; host reorders to [SH, NCLASS]
    outp_d = nc.dram_tensor("outp", [128, SHC * NCLASS], F32,
                            kind="ExternalOutput")
    ls_d = nc.dram_tensor("ls", [128, SHC * NCLASS], F32,
                          kind="ExternalOutput")
    io = (adjT, xT, w0, w1, lw, emb0T_d, emb1T_d, outp_d, ls_d)

    with tile.TileContext(nc) as tc:
        with tc.tile_pool(name="sb", bufs=1) as sb, \
             tc.tile_pool(name="scratch", bufs=2) as scratch, \
             tc.tile_pool(name="psum_acc", bufs=2, space="PSUM") as pacc, \
             tc.tile_pool(name="psum_sm", bufs=3, space="PSUM") as psm, \
             tc.tile_pool(name="dram", bufs=1, space="DRAM") as dram:
            for rep in range(reps):
                if rep:
                    tc.strict_bb_all_engine_barrier()
                _emit_body(nc, tc, sb, scratch, pacc, psm, dram, io, rep)

    nc.compile()
    return nc


def _get_nc(reps=1):
    key = f"nc_{reps}"
    if key not in _NC_CACHE:
        _NC_CACHE[key] = _build(reps)
    return _NC_CACHE[key]


class _Runner:
    """One-time jit of the SPMD NEFF executable; repeat calls just execute.

    Mirrors bass2jax.run_bass_via_pjrt's multi-core path, minus donation,
    so device-resident inputs can be reused across timed calls.
    """

    def __init__(self, nc):
        bass2jax.install_neuronx_cc_hook()
        self.nc = nc
        partition_name = (nc.partition_id_tensor.name
                          if nc.partition_id_tensor else None)
        in_names, out_names, out_avals, zero_outs = [], [], [], []
        for alloc in nc.m.functions[0].allocations:
            if not isinstance(alloc, mybir.MemoryLocationSet):
                continue
            name = alloc.memorylocations[0].name
            if alloc.kind == "ExternalInput":
                if name != partition_name:
                    in_names.append(name)
            elif alloc.kind == "ExternalOutput":
                shape = tuple(alloc.tensor_shape)
                dtype = mybir.dt.np(alloc.dtype)
                out_names.append(name)
                out_avals.append(jax.core.ShapedArray(shape, dtype))
                zero_outs.append(np.zeros(shape, dtype))
        self.n_params = len(in_names)
        self.in_names = list(in_names)
        self.out_names = out_names
        self.out_avals = out_avals
        all_in_names = list(in_names) + list(out_names)
        if partition_name is not None:
            all_in_names.append(partition_name)

        def _body(*args):
            operands = list(args)
            if partition_name is not None:
                operands.append(bass2jax.partition_id_tensor())
            outs = bass2jax._bass_exec_p.bind(
                *operands,
                out_avals=tuple(out_avals),
                in_names=tuple(all_in_names),
                out_names=tuple(out_names),
                lowering_input_output_aliases=(),
                sim_require_finite=True,
                sim_require_nnan=True,
                nc=nc,
            )
            return tuple(outs)

        devices = jax.devices()[:NCORES]
        self.mesh = Mesh(np.asarray(devices), ("core",))
        nspec = self.n_params + len(out_names)
        self.sharding = NamedSharding(self.mesh, PartitionSpec("core"))
        self.fn = jax.jit(
            shard_map(_body, mesh=self.mesh,
                      in_specs=(PartitionSpec("core"),) * nspec,
                      out_specs=(PartitionSpec("core"),) * len(out_names),
                      check_rep=False),
            keep_unused=True,
        )
        self.zero_dev = [
            jax.device_put(
                np.zeros((NCORES * z.shape[0], *z.shape[1:]), z.dtype),
                self.sharding)
            for z in zero_outs
        ]

    def put_inputs(self, in_maps):
        concat = [
            np.concatenate([np.asarray(in_maps[c][name])
                            for c in range(NCORES)], axis=0)
            for name in self.in_names
        ]
        return [jax.device_put(a, self.sharding) for a in concat]

    def execute(self, dev_inputs):
        outs = self.fn(*dev_inputs, *self.zero_dev)
        jax.block_until_ready(outs)
        return outs

    def __call__(self, in_maps):
        outs = self.execute(self.put_inputs(in_maps))
        res = []
        for c in range(NCORES):
            res.append({
                name: np.asarray(outs[i]).reshape(
                    NCORES, *self.out_avals[i].shape)[c]
                for i, name in enumerate(self.out_names)
            })
        return res


def _get_runner(reps=1):
    key = f"runner_{reps}"
    if key not in _NC_CACHE:
        _NC_CACHE[key] = _Runner(_get_nc(reps))
    return _NC_CACHE[key]


def _prep_inputs(x, adj, W0, W1, lin_w, lin_b):
    bf = ml_dtypes.bfloat16
    w0cat = np.transpose(np.asarray(W0, np.float32), (1, 0, 2)).reshape(NFEAT, F)
    w1cat = np.transpose(np.asarray(W1, np.float32), (1, 0, 2)).reshape(F, F)
    lw_aug = np.concatenate(
        [np.asarray(lin_w, np.float32),
         np.asarray(lin_b, np.float32)[None, :]], axis=0)
    w0_b = w0cat.astype(bf)
    w1_b = w1cat.astype(bf)
    lw_b = lw_aug.astype(bf)
    xT = np.ascontiguousarray(np.asarray(x, np.float32).T).astype(bf)
    adj32 = np.asarray(adj, np.float32)
    in_maps = []
    for i in range(NCORES):
        sl = slice(i * SH, (i + 1) * SH)
        in_maps.append({
            "adjT": np.ascontiguousarray(adj32[sl, :].T).astype(bf),
            "xT": np.ascontiguousarray(xT[:, sl]),
            "w0": w0_b, "w1": w1_b, "lw": lw_b,
        })
    return in_maps


def kernel(x, adj, W0, a0, W1, a1, lin_w, lin_b):
    in_maps = _prep_inputs(x, adj, W0, W1, lin_w, lin_b)
    results = None
    for attempt in range(3):
        try:
            results = _get_runner()(in_maps)
            break
        except Exception:
            if attempt == 2:
                raise
            # Device may be wedged from a prior process; reset the PJRT
            # client and rebuild the jit (NEFF compile is cached).
            import jax._src.xla_bridge as _xb
            _NC_CACHE.pop("runner_1", None)
            try:
                _xb._clear_backends()
            except Exception:
                pass
    def unraw(a):  # [128, SHC*NCLASS] -> [SH, NCLASS]
        return a.reshape(128, SHC, NCLASS).transpose(1, 0, 2).reshape(SH, NCLASS)

    emb0 = np.concatenate([r["emb0T"].T for r in results], axis=0)
    emb1 = np.concatenate([r["emb1T"].T for r in results], axis=0)
    outp = np.concatenate([unraw(r["outp"]) for r in results], axis=0)
    ls = np.concatenate([unraw(r["ls"]) for r in results], axis=0)
    return (np.ascontiguousarray(ls), np.ascontiguousarray(emb0),
            np.ascontiguousarray(emb1), np.ascontiguousarray(outp))


# revision 35
# speedup vs baseline: 46.9062x; 46.9062x over previous
"""GAT (nn_GAT_76536317214930) on 8 TRN2 NeuronCores.

The reference's attention softmax is dead code; each layer is
    emb = elu(adj @ (x @ Wcat))        with heads concatenated on features,
then out = elu(emb1) @ lin_w + lin_b and log_softmax.

Sharding: rows (destination nodes) of adj split across 8 cores. Each core
holds adjT shard [8192, 1024] (bf16, SBUF-resident, used by both layers),
computes H = x @ Wcat for its own rows, AllGathers H, then accumulates
P^T = H^T-chunks (stationary) @ adjT-chunks (moving) on the PE.

All feature-major [64, n] intermediates; node-major tiles are produced by
matmuls with the feature-major tensor as lhsT (contracting over features),
so no explicit transposes are needed anywhere.
"""
import numpy as np
import ml_dtypes

import jax
from jax.experimental.shard_map import shard_map
from jax.sharding import Mesh, NamedSharding, PartitionSpec

import concourse.bass as bass
import concourse.bacc as bacc
import concourse.mybir as mybir
import concourse.tile as tile
from concourse import bass2jax

NCORES = 8
N = 8192          # nodes
NFEAT = 512       # input features
F = 64            # NHEADS * NHID = 4*16
NCLASS = 40
SH = N // NCORES  # 1024 rows per core
NCH = N // 128    # 64 contraction chunks (global nodes)
SHC = SH // 128   # 8 node tiles per core shard
XCH = NFEAT // 128  # 4 chunks of input features

BF16 = mybir.dt.bfloat16
FP8 = mybir.dt.float8e4
F32 = mybir.dt.float32
AF = mybir.ActivationFunctionType
ALU = mybir.AluOpType

ADJ_DMA_GROUPS = 16  # adjT loaded in 16 DMAs of 4 chunks (1 MiB each)

_NC_CACHE = {}


def _emit_elu(nc, pool, out_f32, out_alt, src_halves, tag):
    """elu(x) = max(x, exp(min(x, 0)) - 1), per [64, 512] half."""
    for i, (ps, off) in enumerate(src_halves):
        w = ps.shape[-1]
        m_t = pool.tile([F, w], F32, name=f"{tag}_m{i}", tag="elu_m", bufs=2)
        e_t = pool.tile([F, w], F32, name=f"{tag}_e{i}", tag="elu_e", bufs=2)
        nc.vector.tensor_scalar_min(m_t[:], ps, 0.0)
        nc.scalar.activation(e_t[:], m_t[:], AF.Exp)
        nc.vector.tensor_scalar_add(e_t[:], e_t[:], -1.0)
        if out_f32 is not None:
            nc.vector.tensor_tensor(out_f32[:, off:off + w], e_t[:], ps, ALU.max)
        if out_alt is not None:
            nc.vector.tensor_tensor(out_alt[:, off:off + w], e_t[:], ps, ALU.max)


def _emit_body(nc, tc, sb, scratch, pacc, psm, dram, io, rep, probe=None,
               sim1=False):
    """One full forward pass. All tiles tagged so reps share SBUF slots."""
    rg = [list(range(NCORES))]
    adjT, xT, w0, w1, lw, emb0T_d, emb1T_d, outp_d, ls_d = io
    r = f"_{rep}"

    def T(pool, shape, dtype, nm, **kw):
        return pool.tile(shape, dtype, name=nm + r, tag=nm, **kw)

    # ---- persistent SBUF tensors ----
    adjT_sb = T(sb, [128, NCH, SH], FP8, "adjT_sb")
    xT_sb = T(sb, [128, XCH, SH], BF16, "xT_sb")
    w0_sb = T(sb, [128, XCH, F], BF16, "w0_sb")
    w1_sb = T(sb, [F, F], BF16, "w1_sb")
    lw_sb = T(sb, [F + 1, NCLASS], BF16, "lw_sb")
    h0_sb = T(sb, [128, NCH * F], BF16, "h0_sb")
    h1_sb = T(sb, [128, NCH * F], BF16, "h1_sb")
    emb0T = T(sb, [F, SH], F32, "emb0T")
    emb0Tb = T(sb, [F, SH], BF16, "emb0Tb")
    emb1T = T(sb, [F, SH], F32, "emb1T")
    e2T = T(sb, [F + 1, SH], BF16, "e2T")
    h0loc = T(sb, [128, SHC, F], BF16, "h0loc")
    h1loc = T(sb, [128, SHC, F], BF16, "h1loc")
    out_sb = T(sb, [128, SHC, NCLASS], F32, "out_sb")
    t_sb = T(sb, [128, SHC, NCLASS], F32, "t_sb")
    ls_sb = T(sb, [128, SHC, NCLASS], F32, "ls_sb")
    mx_sb = T(sb, [128, SHC], F32, "mx_sb")
    s_sb = T(sb, [128, SHC], F32, "s_sb")
    l_sb = T(sb, [128, SHC], F32, "l_sb")

    # ---- input DMAs (small ones first so the H0 chain overlaps adjT) ----
    xT_r = xT.ap().rearrange("(c p) n -> p c n", p=128)
    for kc in range(XCH):
        nc.sync.dma_start(out=xT_sb[:, kc:kc + 1, :],
                          in_=xT_r[:, kc:kc + 1, :])
    nc.sync.dma_start(out=w0_sb[:],
                      in_=w0.ap().rearrange("(c p) f -> p c f", p=128))
    nc.sync.dma_start(out=w1_sb[:], in_=w1[:])
    nc.sync.dma_start(out=lw_sb[:], in_=lw[:])
    nc.vector.memset(e2T[F:F + 1, :], 1.0)
    adjT_r = adjT.ap().rearrange("(c p) n -> p c n", p=128)
    gsz = NCH // ADJ_DMA_GROUPS
    for g in range(ADJ_DMA_GROUPS):
        nc.sync.dma_start(
            out=adjT_sb[:, g * gsz:(g + 1) * gsz, :],
            in_=adjT_r[:, g * gsz:(g + 1) * gsz, :])
    if probe == "dma":
        return

    # ---- H0 = x @ W0cat (node-major tiles), bounce, AllGather ----
    # Bounce/gather keep the raw SBUF layout [p, c_local*F] so every DMA is
    # large-contiguous; global chunk c = rank*SHC + c_local lines up with
    # node order automatically (node = c*128 + p).
    for m in range(SHC):
        ph0 = psm.tile([128, F], F32, name=f"ph0_{m}{r}", tag="psm")
        for kc in range(XCH):
            nc.tensor.matmul(ph0[:], xT_sb[:, kc, m * 128:(m + 1) * 128],
                             w0_sb[:, kc, :],
                             start=(kc == 0), stop=(kc == XCH - 1))
        nc.scalar.activation(h0loc[:, m, :], ph0[:], AF.Copy)
    h0_bounce = T(dram, [128, SHC * F], BF16, "h0_bounce")
    h0_full = T(dram, [NCORES * 128, SHC * F], BF16, "h0_full",
                addr_space="Local" if sim1 else "Shared")
    nc.sync.dma_start(out=h0_bounce[:], in_=h0loc[:])
    if sim1:  # single-core timing model: AG ~= 8 local 128K writes
        for rr in range(NCORES):
            nc.sync.dma_start(out=h0_full[rr * 128:(rr + 1) * 128, :],
                              in_=h0_bounce[:])
    else:
        nc.gpsimd.collective_compute(
            "AllGather", ALU.bypass, replica_groups=rg,
            ins=[h0_bounce[:]], outs=[h0_full[:]])
    h0f_r = h0_full.rearrange("(g p) q -> p g q", p=128)
    h0_sb_r = h0_sb[:].rearrange("p (g q) -> p g q", g=NCORES)
    for g in range(NCORES):
        nc.sync.dma_start(out=h0_sb_r[:, g:g + 1, :],
                          in_=h0f_r[:, g:g + 1, :])
    if probe == "h0":
        return

    # ---- layer 0 big matmul: P0^T[f, n] accumulated over 64 K-chunks.
    # Even chunks hit PE cols 0-63 / PSUM rows 0-63, odd chunks cols 64-127 /
    # rows 64-127 concurrently (col tiling); halves summed in the epilogue.
    p0a = T(pacc, [128, 512], F32, "acc_a")
    p0b = T(pacc, [128, 512], F32, "acc_b")
    for c in range(NCH):
        st, sp = (c < 2), (c >= NCH - 2)
        ro = (c % 2) * F
        tp = (0, ro)
        nc.tensor.matmul(p0a[ro:ro + F, :], h0_sb[:, c * F:(c + 1) * F],
                         adjT_sb[:, c, 0:512], start=st, stop=sp,
                         tile_position=tp)
        nc.tensor.matmul(p0b[ro:ro + F, :], h0_sb[:, c * F:(c + 1) * F],
                         adjT_sb[:, c, 512:SH], start=st, stop=sp,
                         tile_position=tp)
    if probe == "l0mm":
        # consumer so the MM phase isn't dead code under this probe
        dbg = T(sb, [F, 512], F32, "dbg_l0mm")
        nc.scalar.activation(dbg[:], p0a[0:F, :], AF.Copy)
        nc.scalar.activation(dbg[:], p0b[0:F, :], AF.Copy)
        return

    # ---- epilogue-0 + H1 + AllGather, split into node-column halves so the
    # second AG overlaps layer-1's first chunks ----
    HAFL = SHC // 2  # 4 local node tiles per half
    for h, (pp, off) in enumerate([(p0a, 0), (p0b, 512)]):
        hs = f"h{h}"
        s0 = T(sb, [F, 512], F32, f"sum_{hs}")
        t0 = T(sb, [F, 512], F32, f"tmp_{hs}")
        nc.scalar.activation(t0[:], pp[F:128, :], AF.Copy)
        nc.vector.tensor_tensor(s0[:], t0[:], pp[0:F, :], ALU.add)
        _emit_elu(nc, scratch, emb0T[:, off:off + 512], None,
                  [(s0[:], 0)], f"l0{hs}" + r)
        nc.vector.tensor_copy(emb0Tb[:, off:off + 512],
                              emb0T[:, off:off + 512])
        for m in range(h * HAFL, (h + 1) * HAFL):
            ph1 = psm.tile([128, F], F32, name=f"ph1_{m}{r}", tag="psm")
            nc.tensor.matmul(ph1[:], emb0Tb[:, m * 128:(m + 1) * 128],
                             w1_sb[:], start=True, stop=True)
            nc.scalar.activation(h1loc[:, m, :], ph1[:], AF.Copy)
        h1_bounce = T(dram, [128, HAFL * F], BF16, f"h1_bounce{hs}")
        h1_full = T(dram, [NCORES * 128, HAFL * F], BF16, f"h1_full{hs}",
                    addr_space="Local" if sim1 else "Shared")
        nc.sync.dma_start(out=h1_bounce[:],
                            in_=h1loc[:, h * HAFL:(h + 1) * HAFL, :])
        if sim1:
            for rr in range(NCORES):
                nc.sync.dma_start(out=h1_full[rr * 128:(rr + 1) * 128, :],
                                  in_=h1_bounce[:])
        else:
            nc.gpsimd.collective_compute(
                "AllGather", ALU.bypass, replica_groups=rg,
                ins=[h1_bounce[:]], outs=[h1_full[:]])
        # h1_sb free layout: chunk c = g*SHC + cl at cols c*F; half h covers
        # cl in [h*HAFL, (h+1)*HAFL) of every rank g.
        h1f_r = h1_full.rearrange("(g p) q -> p g q", p=128)
        h1_sb_r = h1_sb[:].rearrange(
            "p (g cl q) -> p g cl q", g=NCORES,
            cl=SHC)[:, :, h * HAFL:(h + 1) * HAFL, :]
        for g2 in range(0, NCORES, 2):
            nc.sync.dma_start(out=h1_sb_r[:, g2:g2 + 2, :, :],
                              in_=h1f_r[:, g2:g2 + 2, :])
    nc.sync.dma_start(out=emb0T_d.ap()[:, 0:512], in_=emb0T[:, 0:512])
    nc.sync.dma_start(out=emb0T_d.ap()[:, 512:SH], in_=emb0T[:, 512:SH])
    if probe == "l0":
        return

    # ---- layer 1 big matmul (col-tiled pairing; AG-half A chunks first) ----
    p1a = pacc.tile([128, 512], F32, name=f"acc_a2{r}", tag="acc_a")
    p1b = pacc.tile([128, 512], F32, name=f"acc_b2{r}", tag="acc_b")
    order = [g * SHC + cl for h in range(2)
             for g in range(NCORES)
             for cl in range(h * HAFL, (h + 1) * HAFL)]
    seen = {0: 0, 1: 0}
    for i, c in enumerate(order):
        par = c % 2
        seen[par] += 1
        st = seen[par] == 1
        sp = seen[par] == NCH // 2
        ro = par * F
        tp = (0, ro)
        nc.tensor.matmul(p1a[ro:ro + F, :], h1_sb[:, c * F:(c + 1) * F],
                         adjT_sb[:, c, 0:512], start=st, stop=sp,
                         tile_position=tp)
        nc.tensor.matmul(p1b[ro:ro + F, :], h1_sb[:, c * F:(c + 1) * F],
                         adjT_sb[:, c, 512:SH], start=st, stop=sp,
                         tile_position=tp)
    # ---- per-half: elu -> emb1T; e2 = elu(emb1); classifier; log_softmax ----
    done_l1 = False
    for h, pp in enumerate([p1a, p1b]):
        hs = f"g{h}"
        off = h * 512
        msl = slice(h * HAFL, (h + 1) * HAFL)
        s1 = T(sb, [F, 512], F32, f"sum2_{hs}")
        t1 = T(sb, [F, 512], F32, f"tmp2_{hs}")
        nc.scalar.activation(t1[:], pp[F:128, :], AF.Copy)
        nc.vector.tensor_tensor(s1[:], t1[:], pp[0:F, :], ALU.add)
        _emit_elu(nc, scratch, emb1T[:, off:off + 512],
                  None, [(s1[:], 0)], f"l1{hs}" + r)
        nc.sync.dma_start(out=emb1T_d.ap()[:, off:off + 512],
                          in_=emb1T[:, off:off + 512])
        _emit_elu(nc, scratch, None, e2T[0:F, off:off + 512],
                  [(emb1T[:, off:off + 512], 0)], f"e2{hs}" + r)
        if probe == "l1":
            if h == 1:
                return
            continue

        # classifier tiles for this half
        for m in range(h * HAFL, (h + 1) * HAFL):
            pcls = psm.tile([128, NCLASS], F32, name=f"pcls_{m}{r}", tag="psm")
            nc.tensor.matmul(pcls[:], e2T[:, m * 128:(m + 1) * 128],
                             lw_sb[:], start=True, stop=True)
            nc.scalar.activation(out_sb[:, m, :], pcls[:], AF.Copy)
        nc.sync.dma_start(
            out=outp_d.ap()[:, h * HAFL * NCLASS:(h + 1) * HAFL * NCLASS],
            in_=out_sb[:, msl, :])

        # batched log_softmax over this half's 4 node tiles
        o3 = out_sb[:, msl, :]
        t3 = t_sb[:, msl, :]
        nc.vector.tensor_reduce(mx_sb[:, msl], o3, mybir.AxisListType.X,
                                ALU.max)
        nc.vector.tensor_tensor(
            t3, o3, mx_sb[:, msl].unsqueeze(-1).broadcast_to(
                (128, HAFL, NCLASS)), ALU.subtract)
        e_sm = scratch.tile([128, HAFL, NCLASS], F32, name=f"e_sm{hs}{r}",
                            tag="e_sm", bufs=2)
        nc.scalar.activation(e_sm[:], t3, AF.Exp)
        nc.vector.tensor_reduce(s_sb[:, msl], e_sm[:], mybir.AxisListType.X,
                                ALU.add)
        nc.scalar.activation(l_sb[:, msl], s_sb[:, msl], AF.Ln)
        nc.vector.tensor_tensor(
            ls_sb[:, msl, :], t3, l_sb[:, msl].unsqueeze(-1).broadcast_to(
                (128, HAFL, NCLASS)), ALU.subtract)
        nc.sync.dma_start(
            out=ls_d.ap()[:, h * HAFL * NCLASS:(h + 1) * HAFL * NCLASS],
            in_=ls_sb[:, msl, :])


def _build(reps=1, probe=None, sim1=False):
    nc = bacc.Bacc("TRN2", target_bir_lowering=False, debug=False,
                   num_devices=1 if sim1 else NCORES)

    adjT = nc.dram_tensor("adjT", [N, SH], FP8, kind="ExternalInput")
    xT = nc.dram_tensor("xT", [NFEAT, SH], BF16, kind="ExternalInput")
    w0 = nc.dram_tensor("w0", [NFEAT, F], BF16, kind="ExternalInput")
    w1 = nc.dram_tensor("w1", [F, F], BF16, kind="ExternalInput")
    lw = nc.dram_tensor("lw", [F + 1, NCLASS], BF16, kind="ExternalInput")

    emb0T_d = nc.dram_tensor("emb0T", [F, SH], F32, kind="ExternalOutput")
    emb1T_d = nc.dram_tensor("emb1T", [F, SH], F32, kind="ExternalOutput")
    # raw SBUF layout [p, c_local, class]; host reorders to [SH, NCLASS]
    outp_d = nc.dram_tensor("outp", [128, SHC * NCLASS], F32,
                            kind="ExternalOutput")
    ls_d = nc.dram_tensor("ls", [128, SHC * NCLASS], F32,
                          kind="ExternalOutput")
    io = (adjT, xT, w0, w1, lw, emb0T_d, emb1T_d, outp_d, ls_d)

    with tile.TileContext(nc) as tc:
        with tc.tile_pool(name="sb", bufs=1) as sb, \
             tc.tile_pool(name="scratch", bufs=2) as scratch, \
             tc.tile_pool(name="psum_acc", bufs=2, space="PSUM") as pacc, \
             tc.tile_pool(name="psum_sm", bufs=3, space="PSUM") as psm, \
             tc.tile_pool(name="dram", bufs=1, space="DRAM") as dram:
            for rep in range(reps):
                if rep:
                    tc.strict_bb_all_engine_barrier()
                _emit_body(nc, tc, sb, scratch, pacc, psm, dram, io, rep,
                           probe=probe, sim1=sim1)

    nc.compile()
    return nc


def _get_nc(reps=1, probe=None):
    key = f"nc_{reps}_{probe}"
    if key not in _NC_CACHE:
        _NC_CACHE[key] = _build(reps, probe)
    return _NC_CACHE[key]


class _Runner:
    """One-time jit of the SPMD NEFF executable; repeat calls just execute.

    Mirrors bass2jax.run_bass_via_pjrt's multi-core path, minus donation,
    so device-resident inputs can be reused across timed calls.
    """

    def __init__(self, nc):
        bass2jax.install_neuronx_cc_hook()
        self.nc = nc
        partition_name = (nc.partition_id_tensor.name
                          if nc.partition_id_tensor else None)
        in_names, out_names, out_avals, zero_outs = [], [], [], []
        for alloc in nc.m.functions[0].allocations:
            if not isinstance(alloc, mybir.MemoryLocationSet):
                continue
            name = alloc.memorylocations[0].name
            if alloc.kind == "ExternalInput":
                if name != partition_name:
                    in_names.append(name)
            elif alloc.kind == "ExternalOutput":
                shape = tuple(alloc.tensor_shape)
                dtype = mybir.dt.np(alloc.dtype)
                out_names.append(name)
                out_avals.append(jax.core.ShapedArray(shape, dtype))
                zero_outs.append(np.zeros(shape, dtype))
        self.n_params = len(in_names)
        self.in_names = list(in_names)
        self.out_names = out_names
        self.out_avals = out_avals
        all_in_names = list(in_names) + list(out_names)
        if partition_name is not None:
            all_in_names.append(partition_name)

        def _body(*args):
            operands = list(args)
            if partition_name is not None:
                operands.append(bass2jax.partition_id_tensor())
            outs = bass2jax._bass_exec_p.bind(
                *operands,
                out_avals=tuple(out_avals),
                in_names=tuple(all_in_names),
                out_names=tuple(out_names),
                lowering_input_output_aliases=(),
                sim_require_finite=True,
                sim_require_nnan=True,
                nc=nc,
            )
            return tuple(outs)

        devices = jax.devices()[:NCORES]
        self.mesh = Mesh(np.asarray(devices), ("core",))
        nspec = self.n_params + len(out_names)
        self.sharding = NamedSharding(self.mesh, PartitionSpec("core"))
        self.fn = jax.jit(
            shard_map(_body, mesh=self.mesh,
                      in_specs=(PartitionSpec("core"),) * nspec,
                      out_specs=(PartitionSpec("core"),) * len(out_names),
                      check_rep=False),
            keep_unused=True,
        )
        self.zero_dev = [
            jax.device_put(
                np.zeros((NCORES * z.shape[0], *z.shape[1:]), z.dtype),
                self.sharding)
            for z in zero_outs
        ]

    def put_inputs(self, in_maps):
        concat = [
            np.concatenate([np.asarray(in_maps[c][name])
                            for c in range(NCORES)], axis=0)
            for name in self.in_names
        ]
        return [jax.device_put(a, self.sharding) for a in concat]

    def execute(self, dev_inputs):
        outs = self.fn(*dev_inputs, *self.zero_dev)
        jax.block_until_ready(outs)
        return outs

    def __call__(self, in_maps):
        outs = self.execute(self.put_inputs(in_maps))
        res = []
        for c in range(NCORES):
            res.append({
                name: np.asarray(outs[i]).reshape(
                    NCORES, *self.out_avals[i].shape)[c]
                for i, name in enumerate(self.out_names)
            })
        return res


def _get_runner(reps=1, probe=None):
    key = f"runner_{reps}_{probe}"
    if key not in _NC_CACHE:
        _NC_CACHE[key] = _Runner(_get_nc(reps, probe))
    return _NC_CACHE[key]


_FP8NP = mybir.dt.np(mybir.dt.float8e4)


def _prep_inputs(x, adj, W0, W1, lin_w, lin_b):
    bf = ml_dtypes.bfloat16
    w0cat = np.transpose(np.asarray(W0, np.float32), (1, 0, 2)).reshape(NFEAT, F)
    w1cat = np.transpose(np.asarray(W1, np.float32), (1, 0, 2)).reshape(F, F)
    lw_aug = np.concatenate(
        [np.asarray(lin_w, np.float32),
         np.asarray(lin_b, np.float32)[None, :]], axis=0)
    w0_b = w0cat.astype(bf)
    w1_b = w1cat.astype(bf)
    lw_b = lw_aug.astype(bf)
    xT = np.ascontiguousarray(np.asarray(x, np.float32).T).astype(bf)
    adj32 = np.asarray(adj, np.float32)
    in_maps = []
    for i in range(NCORES):
        sl = slice(i * SH, (i + 1) * SH)
        in_maps.append({
            "adjT": np.ascontiguousarray(adj32[sl, :].T).astype(_FP8NP),
            "xT": np.ascontiguousarray(xT[:, sl]),
            "w0": w0_b, "w1": w1_b, "lw": lw_b,
        })
    return in_maps


def kernel(x, adj, W0, a0, W1, a1, lin_w, lin_b):
    in_maps = _prep_inputs(x, adj, W0, W1, lin_w, lin_b)
    results = None
    for attempt in range(3):
        try:
            results = _get_runner()(in_maps)
            break
        except Exception:
            if attempt == 2:
                raise
            # Device may be wedged from a prior process; reset the PJRT
            # client and rebuild the jit (NEFF compile is cached).
            import jax._src.xla_bridge as _xb
            _NC_CACHE.pop("runner_1_None", None)
            try:
                _xb._clear_backends()
            except Exception:
                pass
    def unraw(a):  # [128, SHC*NCLASS] -> [SH, NCLASS]
        return a.reshape(128, SHC, NCLASS).transpose(1, 0, 2).reshape(SH, NCLASS)

    emb0 = np.concatenate([r["emb0T"].T for r in results], axis=0)
    emb1 = np.concatenate([r["emb1T"].T for r in results], axis=0)
    outp = np.concatenate([unraw(r["outp"]) for r in results], axis=0)
    ls = np.concatenate([unraw(r["ls"]) for r in results], axis=0)
    return (np.ascontiguousarray(ls), np.ascontiguousarray(emb0),
            np.ascontiguousarray(emb1), np.ascontiguousarray(outp))
